# revision 26
# baseline (speedup 1.0000x reference)
"""Trainium2 Bass kernel for nn_DiagonalSSM (LRU-style diagonal complex SSM).

Math: the SSM is linear time-invariant, so y = causal_conv(u, h) with
h[k] = Re(c^H Lam^k b).  Per core (batch-sharded, 32 batches/core) the
4096-step sequence is split into 4 superchunks of L=1024 packed onto the
128 SBUF partitions as (s, b) pairs.  Within a superchunk the causal conv
is computed exactly with block-Toeplitz matmuls (8 distinct 128x128 blocks
of h); cross-superchunk history enters via end-of-superchunk states E
(a matmul against decaying-power matrix P2), a 3-step associative
combination on VectorE, and a projection of initial states through G
accumulated straight into the conv PSUM banks.
"""
import numpy as np

import concourse.bass as bass
import concourse.mybir as mybir
import concourse.tile as tile
from concourse import bacc
from concourse.bass_utils import run_bass_kernel_spmd
from concourse.masks import make_identity

B, T, N = 256, 4096, 64
L = 1024            # superchunk length
S = 4               # superchunks packed on partitions
NB = 8              # 128-blocks per superchunk
BLOC = B // 8       # batches per core
NC = 8

F32 = mybir.dt.float32
F32R = mybir.dt.float32r
# float32r streams at 1 cycle/row (vs 4 for float32) when the moving dim is
# >=256; numerics validated against the reference in test.py.
USE_F32R = True
DT_MM = F32R if USE_F32R else F32

_BUILT = {}


def _build_module():
    if "nc" in _BUILT:
        return _BUILT["nc"]
    nc = bacc.Bacc("TRN2", target_bir_lowering=False, debug=False, num_devices=NC)
    ut = nc.dram_tensor("ut", [128, NB * 128], DT_MM, kind="ExternalInput").ap()
    toep = nc.dram_tensor("toep", [128, NB * 128], DT_MM,
                          kind="ExternalInput").ap()
    p2sb = nc.dram_tensor("p2sb", [128, NB * 128], F32,
                          kind="ExternalInput").ap()
    g = nc.dram_tensor("g", [128, L], DT_MM, kind="ExternalInput").ap()
    vecs = nc.dram_tensor("vecs", [1, 512], F32, kind="ExternalInput").ap()
    y = nc.dram_tensor("y", [128, L], F32, kind="ExternalOutput").ap()

    with tile.TileContext(nc) as tc:
        with (
            tc.tile_pool(name="sb", bufs=1) as sb,
            tc.tile_pool(name="ps", bufs=1, space="PSUM") as ps,
        ):
            # ---- loads, split across the two HWDGE rings ----
            t_ut = sb.tile([128, NB * 128], DT_MM)
            t_toep = sb.tile([128, NB * 128], DT_MM)
            t_p2 = sb.tile([128, NB * 128], F32)
            t_g = sb.tile([128, L], DT_MM)
            t_vecs = sb.tile([1, 512], F32)
            nc.sync.dma_start(t_vecs[:, :], vecs[:, :])
            nc.sync.dma_start(t_ut[:, 0:512], ut[:, 0:512])
            nc.sync.dma_start(t_ut[:, 512:1024], ut[:, 512:1024])
            nc.sync.dma_start(t_toep[:, 0:512], toep[:, 0:512])
            nc.sync.dma_start(t_toep[:, 512:1024], toep[:, 512:1024])
            nc.scalar.dma_start(t_p2[:, 0:512], p2sb[:, 0:512])
            nc.scalar.dma_start(t_p2[:, 512:1024], p2sb[:, 512:1024])
            nc.scalar.dma_start(t_g[:, :], g[:, :])

            # ---- PE warm-up on junk data (no DMA dependency) so the HAM
            # clock gate reaches full speed before the real matmuls ----
            t_ones = sb.tile([1, 128], F32)
            nc.vector.memset(t_ones[:, :], 1.0)
            t_junk = sb.tile([1, 512], F32)
            nc.vector.memset(t_junk[:, :], 1.0)
            p_bc = ps.tile([128, 512], F32)
            for _ in range(2):
                nc.tensor.matmul(p_bc[:, :], t_ones[:, :], t_junk[:, :],
                                 start=True, stop=True)
            # broadcast scan constants to 128 partitions via PE:
            # vecs = [LreF_d | C2_d] for d=2,3 (cmult consts of Lam^(1024(d-1)))
            nc.tensor.matmul(p_bc[:, :], t_ones[:, :], t_vecs[:, :],
                             start=True, stop=True)
            t_bc = sb.tile([128, 512], F32)
            nc.vector.tensor_copy(t_bc[:, :], p_bc[:, :])

            # ---- end-state matmuls first: the scan overlaps the conv ----
            # (fp32: at 128-wide the f32r fast mode doesn't apply anyway)
            p_y = ps.tile([128, L], F32)           # 2 banks
            p_e = ps.tile([128, 128], F32)
            for jb in range(NB):
                nc.tensor.matmul(
                    p_e[:, :], t_ut[:, 128 * jb:128 * (jb + 1)].bitcast(F32),
                    t_p2[:, 128 * jb:128 * (jb + 1)],
                    start=(jb == 0), stop=(jb == NB - 1), skip_group_check=True)

            # ---- Toeplitz conv matmuls ----
            for jb in range(NB):
                lhs = t_ut[:, 128 * jb:128 * (jb + 1)]
                n_a = 4 - jb
                if n_a > 0:
                    nc.tensor.matmul(
                        p_y[:, 128 * jb:512], lhs, t_toep[:, 0:n_a * 128],
                        start=(jb == 0), stop=False, skip_group_check=True)
                lo = max(n_a, 0) * 128
                nc.tensor.matmul(
                    p_y[:, max(512, 128 * jb):1024], lhs,
                    t_toep[:, lo:(8 - jb) * 128],
                    start=(jb == 0), stop=False, skip_group_check=True)

            # ---- scan: X[s] = sum_d Lam^(1024 d) o E[s-d] ----
            # All DVE ops stay partition-aligned; the cross-partition shift
            # (s-d -> s, i.e. down 32d partitions) is done by SBUF->SBUF DMA
            # into disjoint free-slices of a staging tile.
            t_e = sb.tile([128, 128], F32)
            nc.vector.tensor_copy(t_e[:, :], p_e[:, :])
            t_esw = sb.tile([128, 128], F32)
            nc.vector.tensor_copy(t_esw[:, 0:64], t_e[:, 64:128])
            nc.vector.tensor_copy(t_esw[:, 64:128], t_e[:, 0:64])

            t_sh = sb.tile([128, 384], F32)
            nc.vector.memset(t_sh[:, :], 0.0)
            t_m1 = sb.tile([128, 128], F32)
            t_m2 = sb.tile([128, 128], F32)
            nc.gpsimd.dma_start(t_sh[32:128, 0:128], t_e[0:96, :])
            for d in (2, 3):
                lre_d = t_bc[:, (d - 2) * 256:(d - 2) * 256 + 128]
                c2_d = t_bc[:, (d - 2) * 256 + 128:(d - 2) * 256 + 256]
                t_cd = sb.tile([128, 128], F32, name=f"t_c{d}")
                nc.vector.tensor_mul(t_m1[:, :], t_e[:, :], lre_d)
                nc.vector.tensor_mul(t_m2[:, :], t_esw[:, :], c2_d)
                nc.vector.tensor_add(t_cd[:, :], t_m1[:, :], t_m2[:, :])
                nc.gpsimd.dma_start(t_sh[32 * d:128, (d - 1) * 128:d * 128],
                                    t_cd[0:128 - 32 * d, :])
            t_x = sb.tile([128, 128], F32)
            nc.vector.tensor_add(t_x[:, :], t_sh[:, 0:128], t_sh[:, 128:256])
            nc.vector.tensor_add(t_x[:, :], t_x[:, :], t_sh[:, 256:384])

            # ---- transpose X, project through G into conv PSUM ----
            t_id = sb.tile([128, 128], F32)
            make_identity(nc, t_id[:, :])
            p_xt = ps.tile([128, 128], F32)
            nc.tensor.transpose(p_xt[:, :], t_x[:, :], t_id[:, :])
            t_xt = sb.tile([128, 128], DT_MM)
            nc.vector.tensor_copy(t_xt[:, :], p_xt[:, :])
            nc.tensor.matmul(p_y[:, 0:512], t_xt[:, :], t_g[:, 0:512],
                             start=False, stop=False, skip_group_check=True)
            nc.tensor.matmul(p_y[:, 512:1024], t_xt[:, :], t_g[:, 512:1024],
                             start=False, stop=True, skip_group_check=True)

            # ---- evacuate + store ----
            t_y = sb.tile([128, L], F32)
            nc.vector.tensor_copy(t_y[:, 0:512], p_y[:, 0:512])
            nc.vector.tensor_copy(t_y[:, 512:1024], p_y[:, 512:1024])
            nc.sync.dma_start(y[:, 0:512], t_y[:, 0:512])
            nc.scalar.dma_start(y[:, 512:1024], t_y[:, 512:1024])

    nc.compile()
    _BUILT["nc"] = nc
    return nc


def _make_consts(rho, theta, b_real, b_imag, c_real, c_imag):
    rho = np.asarray(rho, np.float64)
    theta = np.asarray(theta, np.float64)
    r = np.exp(-np.logaddexp(0.0, rho))
    lam = r * np.exp(1j * theta)
    b = np.asarray(b_real, np.float64) + 1j * np.asarray(b_imag, np.float64)
    cconj = np.asarray(c_real, np.float64) - 1j * np.asarray(c_imag, np.float64)

    K = 2 * L + 1
    lp = np.empty((K, N), np.complex128)
    lp[0] = 1.0
    for k in range(1, K):
        lp[k] = lp[k - 1] * lam

    h = np.real((cconj * b)[None, :] * lp[:L]).sum(axis=1)

    TOEP = np.zeros((128, NB * 128), np.float64)
    jj = np.arange(128)
    for d in range(NB):
        idx = 128 * d + jj[None, :] - jj[:, None]
        TOEP[:, d * 128:(d + 1) * 128] = np.where(
            idx >= 0, h[np.clip(idx, 0, L - 1)], 0.0)

    P2 = np.empty((L, 128), np.float64)
    bl = b[None, :] * lp[L - 1 - np.arange(L)]
    P2[:, :64] = bl.real
    P2[:, 64:] = bl.imag
    P2SB = P2.reshape(NB, 128, 128).transpose(1, 0, 2).reshape(128, NB * 128)

    gl = cconj[None, :] * lp[1:L + 1]
    G = np.empty((128, L), np.float64)
    G[:64, :] = gl.real.T
    G[64:, :] = -gl.imag.T

    parts = []
    for d in (2, 3):
        ld = lp[L] ** (d - 1)
        parts += [ld.real, ld.real, -ld.imag, ld.imag]
    vecs = np.concatenate(parts).reshape(1, 512)

    f = lambda x: np.ascontiguousarray(x, np.float32)
    return f(TOEP), f(P2SB), f(G), f(vecs)


def kernel(u, rho, theta, b_real, b_imag, c_real, c_imag):
    u = np.asarray(u, np.float32)
    TOEP, P2SB, G, VECS = _make_consts(rho, theta, b_real, b_imag,
                                       c_real, c_imag)
    nc = _build_module()

    in_maps = []
    for c in range(NC):
        uc = u[c * BLOC:(c + 1) * BLOC]                  # (32, 4096)
        utc = np.ascontiguousarray(
            uc.reshape(BLOC, S, NB, 128).transpose(3, 2, 1, 0).reshape(128, NB * 128))
        in_maps.append({"ut": utc, "toep": TOEP, "p2sb": P2SB,
                        "g": G, "vecs": VECS})

    res = run_bass_kernel_spmd(nc, in_maps, core_ids=list(range(NC)))

    out = np.empty((B, T), np.float32)
    for c in range(NC):
        yc = res.results[c]["y"]                         # (128, 1024)
        out[c * BLOC:(c + 1) * BLOC] = (
            yc.reshape(S, BLOC, L).transpose(1, 0, 2).reshape(BLOC, T))
    return out


# revision 28
# speedup vs baseline: 1.3195x; 1.3195x over previous
"""Trainium2 Bass kernel for nn_DiagonalSSM (LRU-style diagonal complex SSM).

Math: the SSM is linear time-invariant, so y = causal_conv(u, h) with
h[k] = Re(c^H Lam^k b).  Per core (batch-sharded, 32 batches/core) the
4096-step sequence is split into 4 superchunks of L=1024 packed onto the
128 SBUF partitions as (s, b) pairs.  Within a superchunk the causal conv
is computed exactly with block-Toeplitz matmuls (8 distinct 128x128 blocks
of h); cross-superchunk history enters via end-of-superchunk states E
(a matmul against decaying-power matrix P2), a 3-step associative
combination on VectorE, and a projection of initial states through G
accumulated straight into the conv PSUM banks.
"""
import numpy as np

import concourse.bass as bass
import concourse.mybir as mybir
import concourse.tile as tile
from concourse import bacc
from concourse.bass_utils import run_bass_kernel_spmd
from concourse.masks import make_identity

B, T, N = 256, 4096, 64
L = 1024            # superchunk length
S = 4               # superchunks packed on partitions
NB = 8              # 128-blocks per superchunk
BLOC = B // 8       # batches per core
NC = 8

F32 = mybir.dt.float32
F32R = mybir.dt.float32r
# float32r streams at 1 cycle/row (vs 4 for float32) when the moving dim is
# >=256; numerics validated against the reference in test.py.
USE_F32R = True
DT_MM = F32R if USE_F32R else F32

_BUILT = {}


def _build_module():
    if "nc" in _BUILT:
        return _BUILT["nc"]
    nc = bacc.Bacc("TRN2", target_bir_lowering=False, debug=False, num_devices=NC)
    ut = nc.dram_tensor("ut", [128, NB * 128], DT_MM, kind="ExternalInput").ap()
    toep = nc.dram_tensor("toep", [128, NB * 128], DT_MM,
                          kind="ExternalInput").ap()
    p2sb = nc.dram_tensor("p2sb", [128, NB * 128], DT_MM,
                          kind="ExternalInput").ap()
    g = nc.dram_tensor("g", [128, L], DT_MM, kind="ExternalInput").ap()
    vecs = nc.dram_tensor("vecs", [1, 512], F32, kind="ExternalInput").ap()
    y = nc.dram_tensor("y", [128, L], F32, kind="ExternalOutput").ap()

    with tile.TileContext(nc) as tc:
        with (
            tc.tile_pool(name="sb", bufs=1) as sb,
            tc.tile_pool(name="ps", bufs=1, space="PSUM") as ps,
        ):
            # ---- loads, split across the two HWDGE rings ----
            t_ut = sb.tile([128, NB * 128], DT_MM)
            t_toep = sb.tile([128, NB * 128], DT_MM)
            t_p2 = sb.tile([128, NB * 128], DT_MM)
            t_g = sb.tile([128, L], DT_MM)
            t_vecs = sb.tile([1, 512], F32)
            nc.sync.dma_start(t_vecs[:, :], vecs[:, :])
            nc.sync.dma_start(t_ut[:, :], ut[:, :])
            nc.sync.dma_start(t_toep[:, :], toep[:, :])
            nc.scalar.dma_start(t_p2[:, :], p2sb[:, :])
            nc.scalar.dma_start(t_g[:, :], g[:, :])

            # ---- PE warm-up on junk data (no DMA dependency) so the HAM
            # clock gate reaches full speed before the real matmuls ----
            t_ones = sb.tile([1, 128], F32)
            nc.vector.memset(t_ones[:, :], 1.0)
            t_junk_f = sb.tile([1, 512], F32)
            nc.vector.memset(t_junk_f[:, :], 1.0)
            t_junk = sb.tile([1, 512], DT_MM)
            nc.vector.tensor_copy(t_junk[:, :], t_junk_f[:, :])
            t_ones_r = sb.tile([1, 128], DT_MM)
            nc.vector.tensor_copy(t_ones_r[:, :], t_ones[:, :])
            p_bc = ps.tile([128, 512], F32)
            for _ in range(4):
                nc.tensor.matmul(p_bc[:, :], t_ones_r[:, :], t_junk[:, :],
                                 start=True, stop=True)
            # broadcast scan constants to 128 partitions via PE:
            # vecs = [LreF_d | C2_d] for d=2,3 (cmult consts of Lam^(1024(d-1)))
            nc.tensor.matmul(p_bc[:, :], t_ones[:, :], t_vecs[:, :],
                             start=True, stop=True)
            t_bc = sb.tile([128, 512], F32)
            nc.vector.tensor_copy(t_bc[:, :], p_bc[:, :])

            # ---- end-state matmuls first: the scan overlaps the conv ----
            # (fp32: at 128-wide the f32r fast mode doesn't apply anyway)
            p_y = ps.tile([128, L], F32)           # 2 banks
            p_e = ps.tile([128, 128], F32)
            for jb in range(NB):
                nc.tensor.matmul(
                    p_e[:, :], t_ut[:, 128 * jb:128 * (jb + 1)],
                    t_p2[:, 128 * jb:128 * (jb + 1)],
                    start=(jb == 0), stop=(jb == NB - 1), skip_group_check=True)

            # ---- Toeplitz conv matmuls ----
            for jb in range(NB):
                lhs = t_ut[:, 128 * jb:128 * (jb + 1)]
                n_a = 4 - jb
                if n_a > 0:
                    nc.tensor.matmul(
                        p_y[:, 128 * jb:512], lhs, t_toep[:, 0:n_a * 128],
                        start=(jb == 0), stop=False, skip_group_check=True)
                lo = max(n_a, 0) * 128
                nc.tensor.matmul(
                    p_y[:, max(512, 128 * jb):1024], lhs,
                    t_toep[:, lo:(8 - jb) * 128],
                    start=(jb == 0), stop=False, skip_group_check=True)

            # ---- scan: X[s] = sum_d Lam^(1024 d) o E[s-d] ----
            # All DVE ops stay partition-aligned; the cross-partition shift
            # (s-d -> s, i.e. down 32d partitions) is done by SBUF->SBUF DMA
            # into disjoint free-slices of a staging tile.
            t_e = sb.tile([128, 128], F32)
            nc.vector.tensor_copy(t_e[:, :], p_e[:, :])
            t_esw = sb.tile([128, 128], F32)
            nc.vector.tensor_copy(t_esw[:, 0:64], t_e[:, 64:128])
            nc.vector.tensor_copy(t_esw[:, 64:128], t_e[:, 0:64])

            t_sh = sb.tile([128, 384], F32)
            nc.vector.memset(t_sh[:, :], 0.0)
            t_m1 = sb.tile([128, 128], F32)
            t_m2 = sb.tile([128, 128], F32)
            nc.gpsimd.dma_start(t_sh[32:128, 0:128], t_e[0:96, :])
            for d in (2, 3):
                lre_d = t_bc[:, (d - 2) * 256:(d - 2) * 256 + 128]
                c2_d = t_bc[:, (d - 2) * 256 + 128:(d - 2) * 256 + 256]
                t_cd = sb.tile([128, 128], F32, name=f"t_c{d}")
                nc.vector.tensor_mul(t_m1[:, :], t_e[:, :], lre_d)
                nc.vector.tensor_mul(t_m2[:, :], t_esw[:, :], c2_d)
                nc.vector.tensor_add(t_cd[:, :], t_m1[:, :], t_m2[:, :])
                nc.gpsimd.dma_start(t_sh[32 * d:128, (d - 1) * 128:d * 128],
                                    t_cd[0:128 - 32 * d, :])
            t_x = sb.tile([128, 128], F32)
            nc.vector.tensor_add(t_x[:, :], t_sh[:, 0:128], t_sh[:, 128:256])
            nc.vector.tensor_add(t_x[:, :], t_x[:, :], t_sh[:, 256:384])

            # ---- transpose X, project through G into conv PSUM ----
            t_id = sb.tile([128, 128], F32)
            make_identity(nc, t_id[:, :])
            p_xt = ps.tile([128, 128], F32)
            nc.tensor.transpose(p_xt[:, :], t_x[:, :], t_id[:, :])
            t_xt = sb.tile([128, 128], DT_MM)
            nc.vector.tensor_copy(t_xt[:, :], p_xt[:, :])
            nc.tensor.matmul(p_y[:, 0:512], t_xt[:, :], t_g[:, 0:512],
                             start=False, stop=False, skip_group_check=True)
            nc.tensor.matmul(p_y[:, 512:1024], t_xt[:, :], t_g[:, 512:1024],
                             start=False, stop=True, skip_group_check=True)

            # ---- evacuate + store ----
            t_y = sb.tile([128, L], F32)
            nc.vector.tensor_copy(t_y[:, 0:512], p_y[:, 0:512])
            nc.vector.tensor_copy(t_y[:, 512:1024], p_y[:, 512:1024])
            nc.sync.dma_start(y[:, 0:512], t_y[:, 0:512])
            nc.scalar.dma_start(y[:, 512:1024], t_y[:, 512:1024])

    nc.compile()
    _BUILT["nc"] = nc
    return nc


def _make_consts(rho, theta, b_real, b_imag, c_real, c_imag):
    rho = np.asarray(rho, np.float64)
    theta = np.asarray(theta, np.float64)
    r = np.exp(-np.logaddexp(0.0, rho))
    lam = r * np.exp(1j * theta)
    b = np.asarray(b_real, np.float64) + 1j * np.asarray(b_imag, np.float64)
    cconj = np.asarray(c_real, np.float64) - 1j * np.asarray(c_imag, np.float64)

    K = 2 * L + 1
    lp = np.empty((K, N), np.complex128)
    lp[0] = 1.0
    for k in range(1, K):
        lp[k] = lp[k - 1] * lam

    h = np.real((cconj * b)[None, :] * lp[:L]).sum(axis=1)

    TOEP = np.zeros((128, NB * 128), np.float64)
    jj = np.arange(128)
    for d in range(NB):
        idx = 128 * d + jj[None, :] - jj[:, None]
        TOEP[:, d * 128:(d + 1) * 128] = np.where(
            idx >= 0, h[np.clip(idx, 0, L - 1)], 0.0)

    P2 = np.empty((L, 128), np.float64)
    bl = b[None, :] * lp[L - 1 - np.arange(L)]
    P2[:, :64] = bl.real
    P2[:, 64:] = bl.imag
    P2SB = P2.reshape(NB, 128, 128).transpose(1, 0, 2).reshape(128, NB * 128)

    gl = cconj[None, :] * lp[1:L + 1]
    G = np.empty((128, L), np.float64)
    G[:64, :] = gl.real.T
    G[64:, :] = -gl.imag.T

    parts = []
    for d in (2, 3):
        ld = lp[L] ** (d - 1)
        parts += [ld.real, ld.real, -ld.imag, ld.imag]
    vecs = np.concatenate(parts).reshape(1, 512)

    f = lambda x: np.ascontiguousarray(x, np.float32)
    return f(TOEP), f(P2SB), f(G), f(vecs)


def kernel(u, rho, theta, b_real, b_imag, c_real, c_imag):
    u = np.asarray(u, np.float32)
    TOEP, P2SB, G, VECS = _make_consts(rho, theta, b_real, b_imag,
                                       c_real, c_imag)
    nc = _build_module()

    in_maps = []
    for c in range(NC):
        uc = u[c * BLOC:(c + 1) * BLOC]                  # (32, 4096)
        utc = np.ascontiguousarray(
            uc.reshape(BLOC, S, NB, 128).transpose(3, 2, 1, 0).reshape(128, NB * 128))
        in_maps.append({"ut": utc, "toep": TOEP, "p2sb": P2SB,
                        "g": G, "vecs": VECS})

    res = run_bass_kernel_spmd(nc, in_maps, core_ids=list(range(NC)))

    out = np.empty((B, T), np.float32)
    for c in range(NC):
        yc = res.results[c]["y"]                         # (128, 1024)
        out[c * BLOC:(c + 1) * BLOC] = (
            yc.reshape(S, BLOC, L).transpose(1, 0, 2).reshape(BLOC, T))
    return out
